# revision 15
# baseline (speedup 1.0000x reference)
"""Trainium2 Bass kernel for BatchedAttentionBlock over ragged graph batches.

Contract: kernel(**inputs) takes FULL unsharded inputs (x [N,256] float32,
batch [N] int32 sorted graph ids, attention/norm params) and returns the FULL
[N,256] float32 output, computing on 8 NeuronCores via run_bass_kernel_spmd.

Distribution: 64 graphs sorted by length; slot s holds length-ranks
[8s, 8s+8) — one graph per core per slot. A single SPMD program is built with
per-slot lengths Ls = max length within the slot, so per-core shapes agree
while staying close to the true ragged sizes (slack <= a few nodes, handled by
a -inf key mask folded into the softmax exp).

Per-graph pipeline on device (T layout: channels on partitions, seq on free):
  qkT = Wqk^T.T @ xT          (PE, PSUM)  -> +bias evac to SBUF (DVE)
  v   = xT.T @ WvT            (PE)        -> +bias evac (DVE)   [normal layout]
  per head-group g2 (4 heads, row-packed tile_position):
    scT_h = kT_h.T @ qT_h     (PE, 4 PSUM banks)
    P_h   = exp(SCALE*scT + keymask)  (ACT, fused mask, 4 heads/instr)
    den_h = ones.T @ P_h      (PE, col-packed, accumulated over key tiles)
    ctx_h = v_h.T @ P_h       (PE, col-packed, accumulated)
    recipT = approx_recip(den) (DVE)  -> per-head partition broadcast (GPSIMD)
    ctxN  = ctx * recipB      (DVE, evacuates PSUM)
  outT = WoT.T @ ctxN         (PE)
  h = outT + xT (+b) ; per-channel sums & sumsq  (DVE tensor_tensor_reduce)
graph-LN batched across the 8 slots (one cross-partition ones-matmul), then
one erf-GELU per (slot, channel-tile) with LN folded into ACT scale/bias.
"""

import os
import numpy as np

EMBED = 256
NUM_HEADS = 8
HEAD_DIM = 32
NUM_GRAPHS = 64
MAX_LEN = 512
EPS = 1e-5
SCALE = float(1.0 / np.sqrt(HEAD_DIM))
N_CORES = 8
G = NUM_GRAPHS // N_CORES  # graphs (slots) per core
MASK_NEG = -60.0

_CACHE: dict = {}


# --------------------------------------------------------------------------- #
# Bass program
# --------------------------------------------------------------------------- #

def _build_program(Ls: tuple, ob_zero: bool):
    import concourse.bass as bass
    import concourse.tile as tile
    from concourse import bacc, mybir
    from contextlib import ExitStack

    fp32 = mybir.dt.float32
    AF = mybir.ActivationFunctionType
    OP = mybir.AluOpType

    f32r = mybir.dt.float32r

    nc = bacc.Bacc()

    # ---- DRAM I/O ----
    xt_d = nc.declare_dram_parameter("xt", [G, 128, 2, MAX_LEN], f32r, isOutput=False)
    knegb_d = nc.declare_dram_parameter("knegb", [128, G * 4], fp32, isOutput=False)
    invn_d = nc.declare_dram_parameter("invn", [1, G], fp32, isOutput=False)
    bcorr_d = nc.declare_dram_parameter("bcorr", [1, G], fp32, isOutput=False)
    wqkT_d = nc.declare_dram_parameter("wqkT", [128, 2, 512], f32r, isOutput=False)
    wvT_d = nc.declare_dram_parameter("wvT", [128, 2, 256], f32r, isOutput=False)
    woT_d = nc.declare_dram_parameter("woT", [128, 2, 256], f32r, isOutput=False)
    qkb_d = nc.declare_dram_parameter("qkb", [128, 4], fp32, isOutput=False)
    vb_d = nc.declare_dram_parameter("vb", [1, 256], fp32, isOutput=False)
    ob_d = nc.declare_dram_parameter("ob", [128, 2], fp32, isOutput=False)
    lnw_d = nc.declare_dram_parameter("lnw", [128, 2], fp32, isOutput=False)
    lnb_d = nc.declare_dram_parameter("lnb", [128, 2], fp32, isOutput=False)
    out_d = nc.declare_dram_parameter("out", [G, 128, 2, MAX_LEN], fp32, isOutput=True)

    with tile.TileContext(nc) as tc, ExitStack() as ctx:
        singles = ctx.enter_context(tc.tile_pool(name="singles", bufs=1))
        xpool = ctx.enter_context(tc.tile_pool(name="xpool", bufs=3))
        qkpool = ctx.enter_context(tc.tile_pool(name="qkpool", bufs=2))
        vpool = ctx.enter_context(tc.tile_pool(name="vpool", bufs=2))
        ppool = ctx.enter_context(tc.tile_pool(name="ppool", bufs=3))
        rpool = ctx.enter_context(tc.tile_pool(name="rpool", bufs=2))
        cpool = ctx.enter_context(tc.tile_pool(name="cpool", bufs=2))
        hpool = ctx.enter_context(tc.tile_pool(name="hpool", bufs=G + 1))
        sqpool = ctx.enter_context(tc.tile_pool(name="sqpool", bufs=2))
        opool = ctx.enter_context(tc.tile_pool(name="opool", bufs=3))
        tiny = ctx.enter_context(tc.tile_pool(name="tiny", bufs=2))

        ps_sc = ctx.enter_context(tc.tile_pool(name="ps_sc", bufs=1, space="PSUM"))
        ps_duo = ctx.enter_context(tc.tile_pool(name="ps_duo", bufs=2, space="PSUM"))

        # ---- load constants/weights ----
        wqkT = singles.tile([128, 2, 512], f32r)
        nc.sync.dma_start(out=wqkT, in_=wqkT_d[:, :, :])
        wvT = singles.tile([128, 2, 256], f32r)
        nc.sync.dma_start(out=wvT, in_=wvT_d[:, :, :])
        woT = singles.tile([128, 2, 256], f32r)
        nc.sync.dma_start(out=woT, in_=woT_d[:, :, :])
        qkb = singles.tile([128, 4], fp32)
        nc.sync.dma_start(out=qkb, in_=qkb_d[:, :])
        ob = singles.tile([128, 2], fp32)
        nc.sync.dma_start(out=ob, in_=ob_d[:, :])
        lnw = singles.tile([128, 2], fp32)
        nc.sync.dma_start(out=lnw, in_=lnw_d[:, :])
        lnb = singles.tile([128, 2], fp32)
        nc.sync.dma_start(out=lnb, in_=lnb_d[:, :])
        knegb = singles.tile([128, G * 4], fp32)
        nc.sync.dma_start(out=knegb, in_=knegb_d[:, :])
        invn = singles.tile([1, G], fp32)
        nc.sync.dma_start(out=invn, in_=invn_d[:, :])
        bcorr = singles.tile([1, G], fp32)
        nc.sync.dma_start(out=bcorr, in_=bcorr_d[:, :])
        vbrow = singles.tile([1, 256], fp32)
        nc.sync.dma_start(out=vbrow, in_=vb_d[:, :])
        vbB = singles.tile([128, 256], fp32)
        nc.gpsimd.partition_broadcast(vbB[:, :], vbrow[:, :], channels=128)
        ones_blk = singles.tile([128, 32], fp32)
        nc.vector.memset(ones_blk, 1.0)
        ones_ls = singles.tile([128, MAX_LEN], fp32)
        nc.vector.memset(ones_ls, 1.0)
        statsAll = singles.tile([128, 4 * G], fp32)

        h_tiles = []

        # ================= per-graph attention =================
        for s in range(G):
            L = int(Ls[s])
            nkt = (L + 127) // 128

            xT = xpool.tile([128, 2, MAX_LEN], f32r, tag="xT")
            nc.sync.dma_start(out=xT[:, :, :L], in_=xt_d[s, :, :, :L])

            # --- qkT projection: [512ch, L] as 2 duo passes of 2 m-tiles ---
            qT = qkpool.tile([128, 2, MAX_LEN], fp32, tag="qT")
            kT = qkpool.tile([128, 2, MAX_LEN], fp32, tag="kT")
            for half, dst in ((0, qT), (1, kT)):
                qk_ps = ps_duo.tile([128, 2, 512], fp32, tag="duo")
                for m2 in range(2):
                    m = 2 * half + m2
                    for kt in range(2):
                        nc.tensor.matmul(
                            qk_ps[:, m2, :L],
                            wqkT[:, kt, bass.ts(m, 128)],
                            xT[:, kt, :L],
                            start=(kt == 0),
                            stop=(kt == 1),
                        )
                    nc.vector.tensor_scalar_add(
                        out=dst[:, m2, :L],
                        in0=qk_ps[:, m2, :L],
                        scalar1=qkb[:, m : m + 1],
                    )

            # --- v projection: normal layout [keys, 256] per key tile ---
            v_ps = ps_duo.tile([128, 4, 256], fp32, tag="duo")
            for kt2 in range(nkt):
                ks = min(128, L - 128 * kt2)
                for kt in range(2):
                    nc.tensor.matmul(
                        v_ps[:ks, kt2, :],
                        xT[:, kt, bass.ds(128 * kt2, ks)],
                        wvT[:, kt, :],
                        start=(kt == 0),
                        stop=(kt == 1),
                    )
            v_sb = vpool.tile([128, 4, 256], fp32, tag="v_sb")
            for kt2 in range(nkt):
                ks = min(128, L - 128 * kt2)
                nc.vector.tensor_tensor(
                    out=v_sb[:ks, kt2, :],
                    in0=v_ps[:ks, kt2, :],
                    in1=vbB[:ks, :],
                    op=OP.add,
                )

            ctxN = cpool.tile([128, 2, MAX_LEN], f32r, tag="ctxN")

            # --- attention per head-group (4 heads row/col-packed) ---
            for g2 in range(2):
                sc_ps = ps_sc.tile([128, 4, 512], fp32, tag="sc")
                ph = ppool.tile([128, 4, MAX_LEN], fp32, tag="ph")
                dn_ps = ps_duo.tile([128, 512], fp32, tag="duo")
                cx_ps = ps_duo.tile([128, 512], fp32, tag="duo")
                for kt2 in range(nkt):
                    ks = min(128, L - 128 * kt2)
                    for hh in range(4):
                        nc.tensor.matmul(
                            sc_ps[:ks, hh, :L],
                            kT[bass.ts(hh, 32), g2, bass.ds(128 * kt2, ks)],
                            qT[bass.ts(hh, 32), g2, :L],
                            start=True,
                            stop=True,
                            tile_position=(32 * hh, 0),
                        )
                    # exp(SCALE*s + key_mask) for 4 heads in one ACT op
                    nc.scalar.activation(
                        out=ph[:ks, :, :L],
                        in_=sc_ps[:ks, :, :L],
                        func=AF.Exp,
                        bias=knegb[:ks, s * 4 + kt2 : s * 4 + kt2 + 1],
                        scale=SCALE,
                    )
                    for hh in range(4):
                        nc.tensor.matmul(
                            dn_ps[bass.ts(hh, 32), :L],
                            ones_blk[:ks, :],
                            ph[:ks, hh, :L],
                            start=(kt2 == 0),
                            stop=(kt2 == nkt - 1),
                            tile_position=(0, 32 * hh),
                            skip_group_check=True,
                        )
                    for hh in range(4):
                        nc.tensor.matmul(
                            cx_ps[bass.ts(hh, 32), :L],
                            v_sb[:ks, kt2, bass.ts(4 * g2 + hh, 32)],
                            ph[:ks, hh, :L],
                            start=(kt2 == 0),
                            stop=(kt2 == nkt - 1),
                            tile_position=(0, 32 * hh),
                            skip_group_check=True,
                        )
                recipT = rpool.tile([128, MAX_LEN], fp32, tag="recipT")
                nc.vector.reciprocal_approx_fast(recipT[:, :L], dn_ps[:, :L])
                nc.vector.tensor_tensor(
                    out=ctxN[:, g2, :L],
                    in0=cx_ps[:, :L],
                    in1=recipT[:, :L],
                    op=OP.mult,
                )

            # --- output projection + residual + stats ---
            op_ps = ps_duo.tile([128, 2, 512], fp32, tag="duo")
            for m2 in range(2):
                for g2 in range(2):
                    nc.tensor.matmul(
                        op_ps[:, m2, :L],
                        woT[:, g2, bass.ts(m2, 128)],
                        ctxN[:, g2, :L],
                        start=(g2 == 0),
                        stop=(g2 == 1),
                    )
            h_sb = hpool.tile([128, 2, MAX_LEN], fp32, tag="h")
            h_tiles.append((h_sb, L))
            hsq = sqpool.tile([128, 2, MAX_LEN], fp32, tag="hsq")
            for m2 in range(2):
                if ob_zero:
                    nc.vector.tensor_tensor(
                        out=h_sb[:, m2, :L],
                        in0=op_ps[:, m2, :L],
                        in1=xT[:, m2, :L].bitcast(fp32),
                        op=OP.add,
                    )
                else:
                    nc.vector.affine_then_add(
                        out=h_sb[:, m2, :L],
                        in0=op_ps[:, m2, :L],
                        in1=xT[:, m2, :L].bitcast(fp32),
                        scale=ones_blk[:, 0:1],
                        bias=ob[:, m2 : m2 + 1],
                    )
                nc.vector.affine_mul_reduce(
                    out=hsq[:, m2, :L],
                    accum_out=statsAll[:, 4 * s + m2 : 4 * s + m2 + 1],
                    in0=h_sb[:, m2, :L],
                    in1=ones_ls[:, :L],
                    scale=1.0,
                    bias=0.0,
                )
                nc.vector.affine_mul_reduce(
                    out=hsq[:, m2, :L],
                    accum_out=statsAll[:, 4 * s + 2 + m2 : 4 * s + 2 + m2 + 1],
                    in0=h_sb[:, m2, :L],
                    in1=h_sb[:, m2, :L],
                    scale=1.0,
                    bias=0.0,
                )

        # ================= graph-LN finalize (batched) =================
        st_ps = ps_duo.tile([1, 4 * G], fp32, tag="duo")
        nc.tensor.matmul(st_ps[:, :], ones_blk[:, 0:1], statsAll[:, :],
                         start=True, stop=True)
        statsv = tiny.tile([1, 4 * G], fp32, tag="statsv")
        nc.vector.tensor_copy(statsv[:, :], st_ps[:, :])
        sv = statsv[:, :].rearrange("p (g k) -> p g k", k=4)
        Ssum = tiny.tile([1, G], fp32, tag="Ssum")
        Qsum = tiny.tile([1, G], fp32, tag="Qsum")
        nc.vector.tensor_tensor(out=Ssum[:, :], in0=sv[:, :, 0], in1=sv[:, :, 1],
                                op=OP.add)
        nc.vector.tensor_tensor(out=Qsum[:, :], in0=sv[:, :, 2], in1=sv[:, :, 3],
                                op=OP.add)
        mean = tiny.tile([1, G], fp32, tag="mean")
        nc.vector.tensor_tensor(out=mean[:, :], in0=Ssum[:, :], in1=invn[:, :],
                                op=OP.mult)
        nc.vector.tensor_tensor(out=mean[:, :], in0=mean[:, :], in1=bcorr[:, :],
                                op=OP.add)
        ex2 = tiny.tile([1, G], fp32, tag="ex2")
        nc.vector.tensor_tensor(out=ex2[:, :], in0=Qsum[:, :], in1=invn[:, :],
                                op=OP.mult)
        msq = tiny.tile([1, G], fp32, tag="msq")
        nc.vector.tensor_tensor(out=msq[:, :], in0=mean[:, :], in1=mean[:, :],
                                op=OP.mult)
        var = tiny.tile([1, G], fp32, tag="var")
        nc.vector.tensor_tensor(out=var[:, :], in0=ex2[:, :], in1=msq[:, :],
                                op=OP.subtract)
        nc.vector.tensor_scalar_add(out=var[:, :], in0=var[:, :], scalar1=EPS)
        std = tiny.tile([1, G], fp32, tag="std")
        nc.scalar.activation(out=std[:, :], in_=var[:, :], func=AF.Sqrt)
        rstd = tiny.tile([1, G], fp32, tag="rstd")
        nc.vector.reciprocal(out=rstd[:, :], in_=std[:, :])

        rstdB = tiny.tile([128, G], fp32, tag="rstdB")
        nc.gpsimd.partition_broadcast(rstdB[:, :], rstd[:, :], channels=128)
        meanB = tiny.tile([128, G], fp32, tag="meanB")
        nc.gpsimd.partition_broadcast(meanB[:, :], mean[:, :], channels=128)

        # per-(channel-tile, graph) gelu scale/bias: scale=rstd*lnw,
        # bias = lnb - scale*(mean - b_out)
        gsc = []
        for m2 in range(2):
            sc_m = tiny.tile([128, G], fp32, tag=f"sc{m2}")
            nc.vector.tensor_scalar_mul(out=sc_m[:, :], in0=rstdB[:, :],
                                        scalar1=lnw[:, m2 : m2 + 1])
            t1 = tiny.tile([128, G], fp32, tag=f"t1{m2}")
            nc.vector.tensor_scalar(
                out=t1[:, :], in0=meanB[:, :], scalar1=ob[:, m2 : m2 + 1],
                scalar2=None, op0=OP.subtract,
            )
            t2 = tiny.tile([128, G], fp32, tag=f"t2{m2}")
            nc.vector.tensor_tensor(out=t2[:, :], in0=sc_m[:, :], in1=t1[:, :],
                                    op=OP.mult)
            bi_m = tiny.tile([128, G], fp32, tag=f"bi{m2}")
            nc.vector.tensor_scalar(
                out=bi_m[:, :], in0=t2[:, :], scalar1=-1.0, scalar2=lnb[:, m2 : m2 + 1],
                op0=OP.mult, op1=OP.add,
            )
            gsc.append((sc_m, bi_m))

        # ================= gelu + store =================
        for s in range(G):
            h_sb, L = h_tiles[s]
            fin = opool.tile([128, 2, MAX_LEN], fp32, tag="fin")
            for m2 in range(2):
                sc_m, bi_m = gsc[m2]
                nc.scalar.activation(
                    out=fin[:, m2, :L],
                    in_=h_sb[:, m2, :L],
                    func=AF.Gelu,
                    bias=bi_m[:, s : s + 1],
                    scale=sc_m[:, s : s + 1],
                )
            nc.sync.dma_start(out=out_d[s, :, :, :L], in_=fin[:, :, :L])

    nc.compile()
    return nc


# --------------------------------------------------------------------------- #
# Host wrapper
# --------------------------------------------------------------------------- #

def kernel(x, batch, in_proj_w, in_proj_b, out_proj_w, out_proj_b,
           ln_weight, ln_bias):
    x = np.asarray(x, dtype=np.float32)
    batch = np.asarray(batch, dtype=np.int32)
    in_proj_w = np.asarray(in_proj_w, dtype=np.float32)
    in_proj_b = np.asarray(in_proj_b, dtype=np.float32)
    out_proj_w = np.asarray(out_proj_w, dtype=np.float32)
    out_proj_b = np.asarray(out_proj_b, dtype=np.float32)
    ln_weight = np.asarray(ln_weight, dtype=np.float32)
    ln_bias = np.asarray(ln_bias, dtype=np.float32)

    N = x.shape[0]
    counts = np.bincount(batch, minlength=NUM_GRAPHS).astype(np.int64)
    starts = np.concatenate([[0], np.cumsum(counts)[:-1]])

    # slot assignment: sort by length desc; slot s <- ranks [8s, 8s+8)
    order = np.argsort(-counts, kind="stable")
    assign = np.empty((N_CORES, G), dtype=np.int64)  # graph id per (core, slot)
    Ls = np.empty(G, dtype=np.int64)
    for s in range(G):
        ranks = order[s * N_CORES : (s + 1) * N_CORES]
        assign[:, s] = ranks
        Ls[s] = min(MAX_LEN, -4 * (-int(counts[ranks].max()) // 4))

    ob_zero = not np.any(out_proj_b != 0.0)
    key = (tuple(int(v) for v in Ls), ob_zero)
    if key not in _CACHE:
        _CACHE[key] = _build_program(key[0], ob_zero)
    nc = _CACHE[key]

    # ---- shared (replicated) weight tensors ----
    wqkT = np.ascontiguousarray(
        in_proj_w[:512].T.reshape(2, 128, 512).transpose(1, 0, 2))
    wvT = np.ascontiguousarray(
        in_proj_w[512:768].T.reshape(2, 128, 256).transpose(1, 0, 2))
    woT = np.ascontiguousarray(
        out_proj_w.T.reshape(2, 128, 256).transpose(1, 0, 2))
    qkb = np.ascontiguousarray(in_proj_b[:512].reshape(4, 128).T)
    vb = np.ascontiguousarray(in_proj_b[512:768][None, :])
    ob = np.ascontiguousarray(out_proj_b.reshape(2, 128).T)
    lnw = np.ascontiguousarray(ln_weight.reshape(2, 128).T)
    lnb = np.ascontiguousarray(ln_bias.reshape(2, 128).T)
    sum_ob = float(out_proj_b.sum())

    in_maps = []
    for c in range(N_CORES):
        xt = np.zeros((G, 128, 2, MAX_LEN), dtype=np.float32)
        knegb = np.zeros((128, G * 4), dtype=np.float32)
        invn = np.zeros((1, G), dtype=np.float32)
        bcorr = np.zeros((1, G), dtype=np.float32)
        for s in range(G):
            g = assign[c, s]
            L = int(counts[g])
            xg = x[starts[g] : starts[g] + L]  # [L, 256]
            xT = xg.T.reshape(2, 128, L).transpose(1, 0, 2)  # [128, 2, L]
            xt[s, :, :, :L] = xT
            nkt = (int(Ls[s]) + 127) // 128
            for kt2 in range(nkt):
                pvalid = np.arange(128) + 128 * kt2 < L
                knegb[:, s * 4 + kt2] = np.where(pvalid, 0.0, MASK_NEG)
            invn[0, s] = 1.0 / (L * EMBED)
            bcorr[0, s] = sum_ob * L * invn[0, s]
        in_maps.append(dict(
            xt=xt, knegb=knegb, invn=invn, bcorr=bcorr,
            wqkT=wqkT, wvT=wvT, woT=woT, qkb=qkb, vb=vb, ob=ob,
            lnw=lnw, lnb=lnb,
        ))

    from concourse.bass_utils import run_bass_kernel_spmd
    res = run_bass_kernel_spmd(nc, in_maps, list(range(N_CORES)))

    out = np.empty((N, EMBED), dtype=np.float32)
    for c in range(N_CORES):
        o = res.results[c]["out"]  # [G, 128, 2, 512]
        for s in range(G):
            g = assign[c, s]
            L = int(counts[g])
            outT = o[s, :, :, :L].transpose(1, 0, 2).reshape(EMBED, L)
            out[starts[g] : starts[g] + L] = outT.T
    return out


# revision 16
# speedup vs baseline: 2.1215x; 2.1215x over previous
"""Trainium2 Bass kernel for BatchedAttentionBlock over ragged graph batches.

Contract: kernel(**inputs) takes FULL unsharded inputs (x [N,256] float32,
batch [N] int32 sorted graph ids, attention/norm params) and returns the FULL
[N,256] float32 output, computing on 8 NeuronCores via run_bass_kernel_spmd.

Distribution: 64 graphs sorted by length; slot s holds length-ranks
[8s, 8s+8) — one graph per core per slot. A single SPMD program is built with
per-slot lengths Ls = max length within the slot, so per-core shapes agree
while staying close to the true ragged sizes (slack <= a few nodes, handled by
a -inf key mask folded into the softmax exp).

Per-graph pipeline on device (T layout: channels on partitions, seq on free):
  qkT = Wqk^T.T @ xT          (PE, PSUM)  -> +bias evac to SBUF (DVE)
  v   = xT.T @ WvT            (PE)        -> +bias evac (DVE)   [normal layout]
  per head-group g2 (4 heads, row-packed tile_position):
    scT_h = kT_h.T @ qT_h     (PE, 4 PSUM banks)
    P_h   = exp(SCALE*scT + keymask)  (ACT, fused mask, 4 heads/instr)
    den_h = ones.T @ P_h      (PE, col-packed, accumulated over key tiles)
    ctx_h = v_h.T @ P_h       (PE, col-packed, accumulated)
    recipT = approx_recip(den) (DVE)  -> per-head partition broadcast (GPSIMD)
    ctxN  = ctx * recipB      (DVE, evacuates PSUM)
  outT = WoT.T @ ctxN         (PE)
  h = outT + xT (+b) ; per-channel sums & sumsq  (DVE tensor_tensor_reduce)
graph-LN batched across the 8 slots (one cross-partition ones-matmul), then
one erf-GELU per (slot, channel-tile) with LN folded into ACT scale/bias.
"""

import os
import numpy as np

EMBED = 256
NUM_HEADS = 8
HEAD_DIM = 32
NUM_GRAPHS = 64
MAX_LEN = 512
EPS = 1e-5
SCALE = float(1.0 / np.sqrt(HEAD_DIM))
N_CORES = 8
G = NUM_GRAPHS // N_CORES  # graphs (slots) per core
MASK_NEG = -60.0

_CACHE: dict = {}


# --------------------------------------------------------------------------- #
# Bass program
# --------------------------------------------------------------------------- #

def _build_program(Ls: tuple, ob_zero: bool):
    import concourse.bass as bass
    import concourse.tile as tile
    from concourse import bacc, mybir
    from contextlib import ExitStack

    fp32 = mybir.dt.float32
    AF = mybir.ActivationFunctionType
    OP = mybir.AluOpType

    f32r = mybir.dt.float32r
    bf16 = mybir.dt.bfloat16

    nc = bacc.Bacc()

    # ---- DRAM I/O ----
    xt_d = nc.declare_dram_parameter("xt", [G, 128, 2, MAX_LEN], f32r, isOutput=False)
    knegb_d = nc.declare_dram_parameter("knegb", [128, G * 4], fp32, isOutput=False)
    invn_d = nc.declare_dram_parameter("invn", [1, G], fp32, isOutput=False)
    bcorr_d = nc.declare_dram_parameter("bcorr", [1, G], fp32, isOutput=False)
    wqkT_d = nc.declare_dram_parameter("wqkT", [128, 2, 512], f32r, isOutput=False)
    wvT_d = nc.declare_dram_parameter("wvT", [128, 2, 256], f32r, isOutput=False)
    woT_d = nc.declare_dram_parameter("woT", [128, 2, 256], f32r, isOutput=False)
    qkb_d = nc.declare_dram_parameter("qkb", [128, 4], fp32, isOutput=False)
    vb_d = nc.declare_dram_parameter("vb", [1, 256], fp32, isOutput=False)
    ob_d = nc.declare_dram_parameter("ob", [128, 2], fp32, isOutput=False)
    lnw_d = nc.declare_dram_parameter("lnw", [128, 2], fp32, isOutput=False)
    lnb_d = nc.declare_dram_parameter("lnb", [128, 2], fp32, isOutput=False)
    out_d = nc.declare_dram_parameter("out", [G, 128, 2, MAX_LEN], fp32, isOutput=True)

    with tile.TileContext(nc) as tc, ExitStack() as ctx:
        singles = ctx.enter_context(tc.tile_pool(name="singles", bufs=1))
        xpool = ctx.enter_context(tc.tile_pool(name="xpool", bufs=3))
        qkpool = ctx.enter_context(tc.tile_pool(name="qkpool", bufs=2))
        vpool = ctx.enter_context(tc.tile_pool(name="vpool", bufs=2))
        ppool = ctx.enter_context(tc.tile_pool(name="ppool", bufs=3))
        rpool = ctx.enter_context(tc.tile_pool(name="rpool", bufs=2))
        cpool = ctx.enter_context(tc.tile_pool(name="cpool", bufs=2))
        hpool = ctx.enter_context(tc.tile_pool(name="hpool", bufs=G + 1))
        sqpool = ctx.enter_context(tc.tile_pool(name="sqpool", bufs=2))
        opool = ctx.enter_context(tc.tile_pool(name="opool", bufs=3))
        tiny = ctx.enter_context(tc.tile_pool(name="tiny", bufs=2))

        ps_sc = ctx.enter_context(tc.tile_pool(name="ps_sc", bufs=1, space="PSUM"))
        ps_duo = ctx.enter_context(tc.tile_pool(name="ps_duo", bufs=2, space="PSUM"))

        # ---- load constants/weights ----
        wqkT = singles.tile([128, 2, 512], f32r)
        nc.sync.dma_start(out=wqkT, in_=wqkT_d[:, :, :])
        wvT = singles.tile([128, 2, 256], f32r)
        nc.sync.dma_start(out=wvT, in_=wvT_d[:, :, :])
        woT = singles.tile([128, 2, 256], f32r)
        nc.sync.dma_start(out=woT, in_=woT_d[:, :, :])
        qkb = singles.tile([128, 4], fp32)
        nc.sync.dma_start(out=qkb, in_=qkb_d[:, :])
        ob = singles.tile([128, 2], fp32)
        nc.sync.dma_start(out=ob, in_=ob_d[:, :])
        lnw = singles.tile([128, 2], fp32)
        nc.sync.dma_start(out=lnw, in_=lnw_d[:, :])
        lnb = singles.tile([128, 2], fp32)
        nc.sync.dma_start(out=lnb, in_=lnb_d[:, :])
        knegb = singles.tile([128, G * 4], fp32)
        nc.sync.dma_start(out=knegb, in_=knegb_d[:, :])
        invn = singles.tile([1, G], fp32)
        nc.sync.dma_start(out=invn, in_=invn_d[:, :])
        bcorr = singles.tile([1, G], fp32)
        nc.sync.dma_start(out=bcorr, in_=bcorr_d[:, :])
        vbrow = singles.tile([1, 256], fp32)
        nc.sync.dma_start(out=vbrow, in_=vb_d[:, :])
        vbB = singles.tile([128, 256], fp32)
        nc.gpsimd.partition_broadcast(vbB[:, :], vbrow[:, :], channels=128)
        ones_blk = singles.tile([128, 32], bf16)
        nc.vector.memset(ones_blk, 1.0)
        ones32 = singles.tile([128, 1], fp32)
        nc.vector.memset(ones32, 1.0)
        ones_ls = singles.tile([128, MAX_LEN], fp32)
        nc.vector.memset(ones_ls, 1.0)
        statsAll = singles.tile([128, 4 * G], fp32)

        h_tiles = []

        # ================= per-graph attention =================
        for s in range(G):
            L = int(Ls[s])
            nkt = (L + 127) // 128

            xT = xpool.tile([128, 2, MAX_LEN], f32r, tag="xT")
            nc.sync.dma_start(out=xT[:, :, :L], in_=xt_d[s, :, :, :L])

            # --- qkT projection: [512ch, L] as 2 duo passes of 2 m-tiles ---
            qT = qkpool.tile([128, 2, MAX_LEN], bf16, tag="qT")
            kT = qkpool.tile([128, 2, MAX_LEN], bf16, tag="kT")
            for half, dst in ((0, qT), (1, kT)):
                qk_ps = ps_duo.tile([128, 2, 512], fp32, tag="duo")
                for m2 in range(2):
                    m = 2 * half + m2
                    for kt in range(2):
                        nc.tensor.matmul(
                            qk_ps[:, m2, :L],
                            wqkT[:, kt, bass.ts(m, 128)],
                            xT[:, kt, :L],
                            start=(kt == 0),
                            stop=(kt == 1),
                        )
                    nc.vector.tensor_scalar_add(
                        out=dst[:, m2, :L],
                        in0=qk_ps[:, m2, :L],
                        scalar1=qkb[:, m : m + 1],
                    )

            # --- v projection: normal layout [keys, 256] per key tile ---
            v_ps = ps_duo.tile([128, 4, 256], fp32, tag="duo")
            for kt2 in range(nkt):
                ks = min(128, L - 128 * kt2)
                for kt in range(2):
                    nc.tensor.matmul(
                        v_ps[:ks, kt2, :],
                        xT[:, kt, bass.ds(128 * kt2, ks)],
                        wvT[:, kt, :],
                        start=(kt == 0),
                        stop=(kt == 1),
                    )
            v_sb = vpool.tile([128, 4, 256], bf16, tag="v_sb")
            for kt2 in range(nkt):
                ks = min(128, L - 128 * kt2)
                nc.vector.tensor_tensor(
                    out=v_sb[:ks, kt2, :],
                    in0=v_ps[:ks, kt2, :],
                    in1=vbB[:ks, :],
                    op=OP.add,
                )

            ctxN = cpool.tile([128, 2, MAX_LEN], f32r, tag="ctxN")

            # --- attention per head-group (4 heads row/col-packed) ---
            for g2 in range(2):
                sc_ps = ps_sc.tile([128, 4, 512], fp32, tag="sc")
                ph = ppool.tile([128, 4, MAX_LEN], bf16, tag="ph")
                dn_ps = ps_duo.tile([128, 512], fp32, tag="duo")
                cx_ps = ps_duo.tile([128, 512], fp32, tag="duo")
                for kt2 in range(nkt):
                    ks = min(128, L - 128 * kt2)
                    for hh in range(4):
                        nc.tensor.matmul(
                            sc_ps[:ks, hh, :L],
                            kT[bass.ts(hh, 32), g2, bass.ds(128 * kt2, ks)],
                            qT[bass.ts(hh, 32), g2, :L],
                            start=True,
                            stop=True,
                            tile_position=(32 * hh, 0),
                        )
                    # exp(SCALE*s + key_mask) for 4 heads in one ACT op
                    nc.scalar.activation(
                        out=ph[:ks, :, :L],
                        in_=sc_ps[:ks, :, :L],
                        func=AF.Exp,
                        bias=knegb[:ks, s * 4 + kt2 : s * 4 + kt2 + 1],
                        scale=SCALE,
                    )
                    for hh in range(4):
                        nc.tensor.matmul(
                            dn_ps[bass.ts(hh, 32), :L],
                            ones_blk[:ks, :],
                            ph[:ks, hh, :L],
                            start=(kt2 == 0),
                            stop=(kt2 == nkt - 1),
                            tile_position=(0, 32 * hh),
                            skip_group_check=True,
                        )
                    for hh in range(4):
                        nc.tensor.matmul(
                            cx_ps[bass.ts(hh, 32), :L],
                            v_sb[:ks, kt2, bass.ts(4 * g2 + hh, 32)],
                            ph[:ks, hh, :L],
                            start=(kt2 == 0),
                            stop=(kt2 == nkt - 1),
                            tile_position=(0, 32 * hh),
                            skip_group_check=True,
                        )
                recipT = rpool.tile([128, MAX_LEN], fp32, tag="recipT")
                nc.vector.reciprocal_approx_fast(recipT[:, :L], dn_ps[:, :L])
                nc.vector.tensor_tensor(
                    out=ctxN[:, g2, :L],
                    in0=cx_ps[:, :L],
                    in1=recipT[:, :L],
                    op=OP.mult,
                )

            # --- output projection + residual + stats ---
            op_ps = ps_duo.tile([128, 2, 512], fp32, tag="duo")
            for m2 in range(2):
                for g2 in range(2):
                    nc.tensor.matmul(
                        op_ps[:, m2, :L],
                        woT[:, g2, bass.ts(m2, 128)],
                        ctxN[:, g2, :L],
                        start=(g2 == 0),
                        stop=(g2 == 1),
                    )
            h_sb = hpool.tile([128, 2, MAX_LEN], fp32, tag="h")
            h_tiles.append((h_sb, L))
            hsq = sqpool.tile([128, 2, MAX_LEN], fp32, tag="hsq")
            for m2 in range(2):
                if ob_zero:
                    nc.vector.tensor_tensor(
                        out=h_sb[:, m2, :L],
                        in0=op_ps[:, m2, :L],
                        in1=xT[:, m2, :L].bitcast(fp32),
                        op=OP.add,
                    )
                else:
                    nc.vector.affine_then_add(
                        out=h_sb[:, m2, :L],
                        in0=op_ps[:, m2, :L],
                        in1=xT[:, m2, :L].bitcast(fp32),
                        scale=ones32[:, :],
                        bias=ob[:, m2 : m2 + 1],
                    )
                nc.vector.affine_mul_reduce(
                    out=hsq[:, m2, :L],
                    accum_out=statsAll[:, 4 * s + m2 : 4 * s + m2 + 1],
                    in0=h_sb[:, m2, :L],
                    in1=ones_ls[:, :L],
                    scale=1.0,
                    bias=0.0,
                )
                nc.vector.affine_mul_reduce(
                    out=hsq[:, m2, :L],
                    accum_out=statsAll[:, 4 * s + 2 + m2 : 4 * s + 2 + m2 + 1],
                    in0=h_sb[:, m2, :L],
                    in1=h_sb[:, m2, :L],
                    scale=1.0,
                    bias=0.0,
                )

        # ================= graph-LN finalize (batched) =================
        st_ps = ps_duo.tile([1, 4 * G], fp32, tag="duo")
        nc.tensor.matmul(st_ps[:, :], ones32[:, :], statsAll[:, :],
                         start=True, stop=True)
        statsv = tiny.tile([1, 4 * G], fp32, tag="statsv")
        nc.vector.tensor_copy(statsv[:, :], st_ps[:, :])
        sv = statsv[:, :].rearrange("p (g k) -> p g k", k=4)
        Ssum = tiny.tile([1, G], fp32, tag="Ssum")
        Qsum = tiny.tile([1, G], fp32, tag="Qsum")
        nc.vector.tensor_tensor(out=Ssum[:, :], in0=sv[:, :, 0], in1=sv[:, :, 1],
                                op=OP.add)
        nc.vector.tensor_tensor(out=Qsum[:, :], in0=sv[:, :, 2], in1=sv[:, :, 3],
                                op=OP.add)
        mean = tiny.tile([1, G], fp32, tag="mean")
        nc.vector.tensor_tensor(out=mean[:, :], in0=Ssum[:, :], in1=invn[:, :],
                                op=OP.mult)
        nc.vector.tensor_tensor(out=mean[:, :], in0=mean[:, :], in1=bcorr[:, :],
                                op=OP.add)
        ex2 = tiny.tile([1, G], fp32, tag="ex2")
        nc.vector.tensor_tensor(out=ex2[:, :], in0=Qsum[:, :], in1=invn[:, :],
                                op=OP.mult)
        msq = tiny.tile([1, G], fp32, tag="msq")
        nc.vector.tensor_tensor(out=msq[:, :], in0=mean[:, :], in1=mean[:, :],
                                op=OP.mult)
        var = tiny.tile([1, G], fp32, tag="var")
        nc.vector.tensor_tensor(out=var[:, :], in0=ex2[:, :], in1=msq[:, :],
                                op=OP.subtract)
        nc.vector.tensor_scalar_add(out=var[:, :], in0=var[:, :], scalar1=EPS)
        std = tiny.tile([1, G], fp32, tag="std")
        nc.scalar.activation(out=std[:, :], in_=var[:, :], func=AF.Sqrt)
        rstd = tiny.tile([1, G], fp32, tag="rstd")
        nc.vector.reciprocal(out=rstd[:, :], in_=std[:, :])

        rstdB = tiny.tile([128, G], fp32, tag="rstdB")
        nc.gpsimd.partition_broadcast(rstdB[:, :], rstd[:, :], channels=128)
        meanB = tiny.tile([128, G], fp32, tag="meanB")
        nc.gpsimd.partition_broadcast(meanB[:, :], mean[:, :], channels=128)

        # per-(channel-tile, graph) gelu scale/bias: scale=rstd*lnw,
        # bias = lnb - scale*(mean - b_out)
        gsc = []
        for m2 in range(2):
            sc_m = tiny.tile([128, G], fp32, tag=f"sc{m2}")
            nc.vector.tensor_scalar_mul(out=sc_m[:, :], in0=rstdB[:, :],
                                        scalar1=lnw[:, m2 : m2 + 1])
            t1 = tiny.tile([128, G], fp32, tag=f"t1{m2}")
            nc.vector.tensor_scalar(
                out=t1[:, :], in0=meanB[:, :], scalar1=ob[:, m2 : m2 + 1],
                scalar2=None, op0=OP.subtract,
            )
            t2 = tiny.tile([128, G], fp32, tag=f"t2{m2}")
            nc.vector.tensor_tensor(out=t2[:, :], in0=sc_m[:, :], in1=t1[:, :],
                                    op=OP.mult)
            bi_m = tiny.tile([128, G], fp32, tag=f"bi{m2}")
            nc.vector.tensor_scalar(
                out=bi_m[:, :], in0=t2[:, :], scalar1=-1.0, scalar2=lnb[:, m2 : m2 + 1],
                op0=OP.mult, op1=OP.add,
            )
            gsc.append((sc_m, bi_m))

        # ================= gelu + store =================
        for s in range(G):
            h_sb, L = h_tiles[s]
            fin = opool.tile([128, 2, MAX_LEN], fp32, tag="fin")
            for m2 in range(2):
                sc_m, bi_m = gsc[m2]
                nc.scalar.activation(
                    out=fin[:, m2, :L],
                    in_=h_sb[:, m2, :L],
                    func=AF.Gelu,
                    bias=bi_m[:, s : s + 1],
                    scale=sc_m[:, s : s + 1],
                )
            nc.sync.dma_start(out=out_d[s, :, :, :L], in_=fin[:, :, :L])

    nc.compile()
    return nc


# --------------------------------------------------------------------------- #
# Host wrapper
# --------------------------------------------------------------------------- #

def kernel(x, batch, in_proj_w, in_proj_b, out_proj_w, out_proj_b,
           ln_weight, ln_bias):
    x = np.asarray(x, dtype=np.float32)
    batch = np.asarray(batch, dtype=np.int32)
    in_proj_w = np.asarray(in_proj_w, dtype=np.float32)
    in_proj_b = np.asarray(in_proj_b, dtype=np.float32)
    out_proj_w = np.asarray(out_proj_w, dtype=np.float32)
    out_proj_b = np.asarray(out_proj_b, dtype=np.float32)
    ln_weight = np.asarray(ln_weight, dtype=np.float32)
    ln_bias = np.asarray(ln_bias, dtype=np.float32)

    N = x.shape[0]
    counts = np.bincount(batch, minlength=NUM_GRAPHS).astype(np.int64)
    starts = np.concatenate([[0], np.cumsum(counts)[:-1]])

    # slot assignment: sort by length desc; slot s <- ranks [8s, 8s+8)
    order = np.argsort(-counts, kind="stable")
    assign = np.empty((N_CORES, G), dtype=np.int64)  # graph id per (core, slot)
    Ls = np.empty(G, dtype=np.int64)
    for s in range(G):
        ranks = order[s * N_CORES : (s + 1) * N_CORES]
        assign[:, s] = ranks
        Ls[s] = min(MAX_LEN, -4 * (-int(counts[ranks].max()) // 4))

    ob_zero = not np.any(out_proj_b != 0.0)
    key = (tuple(int(v) for v in Ls), ob_zero)
    if key not in _CACHE:
        _CACHE[key] = _build_program(key[0], ob_zero)
    nc = _CACHE[key]

    # ---- shared (replicated) weight tensors ----
    wqkT = np.ascontiguousarray(
        in_proj_w[:512].T.reshape(2, 128, 512).transpose(1, 0, 2))
    wvT = np.ascontiguousarray(
        in_proj_w[512:768].T.reshape(2, 128, 256).transpose(1, 0, 2))
    woT = np.ascontiguousarray(
        out_proj_w.T.reshape(2, 128, 256).transpose(1, 0, 2))
    qkb = np.ascontiguousarray(in_proj_b[:512].reshape(4, 128).T)
    vb = np.ascontiguousarray(in_proj_b[512:768][None, :])
    ob = np.ascontiguousarray(out_proj_b.reshape(2, 128).T)
    lnw = np.ascontiguousarray(ln_weight.reshape(2, 128).T)
    lnb = np.ascontiguousarray(ln_bias.reshape(2, 128).T)
    sum_ob = float(out_proj_b.sum())

    in_maps = []
    for c in range(N_CORES):
        xt = np.zeros((G, 128, 2, MAX_LEN), dtype=np.float32)
        knegb = np.zeros((128, G * 4), dtype=np.float32)
        invn = np.zeros((1, G), dtype=np.float32)
        bcorr = np.zeros((1, G), dtype=np.float32)
        for s in range(G):
            g = assign[c, s]
            L = int(counts[g])
            xg = x[starts[g] : starts[g] + L]  # [L, 256]
            xT = xg.T.reshape(2, 128, L).transpose(1, 0, 2)  # [128, 2, L]
            xt[s, :, :, :L] = xT
            nkt = (int(Ls[s]) + 127) // 128
            for kt2 in range(nkt):
                pvalid = np.arange(128) + 128 * kt2 < L
                knegb[:, s * 4 + kt2] = np.where(pvalid, 0.0, MASK_NEG)
            invn[0, s] = 1.0 / (L * EMBED)
            bcorr[0, s] = sum_ob * L * invn[0, s]
        in_maps.append(dict(
            xt=xt, knegb=knegb, invn=invn, bcorr=bcorr,
            wqkT=wqkT, wvT=wvT, woT=woT, qkb=qkb, vb=vb, ob=ob,
            lnw=lnw, lnb=lnb,
        ))

    from concourse.bass_utils import run_bass_kernel_spmd
    res = run_bass_kernel_spmd(nc, in_maps, list(range(N_CORES)))

    out = np.empty((N, EMBED), dtype=np.float32)
    for c in range(N_CORES):
        o = res.results[c]["out"]  # [G, 128, 2, 512]
        for s in range(G):
            g = assign[c, s]
            L = int(counts[g])
            outT = o[s, :, :, :L].transpose(1, 0, 2).reshape(EMBED, L)
            out[starts[g] : starts[g] + L] = outT.T
    return out
